# revision 3
# baseline (speedup 1.0000x reference)
"""nn_ActorNetwork Trainium kernel: 8-way data-parallel over observations.

Sharding: core c owns nodes [c*64K, (c+1)*64K), its 640 dags, 32 observations.
Edges are sharded by src core and sorted by src on the host (index-only prep);
per-depth active-edge compaction uses the bool masks. Message aggregation is
computed on-device as cumsum + boundary differences (host supplies the static
boundary arrays). The per-depth y tables are exchanged between shards via the
host (pure data movement). All float math runs on the NeuronCores.
"""
import numpy as np
import jax
import jax.numpy as jnp
from functools import partial

N = 512_000
E = 4_000_000
DEPTH = 3
NUM_DAGS = 5_120
NODES_PER_DAG = 100
NUM_OBS = 256
DAGS_PER_OBS = 20
DIM = 16
NUM_EXEC = 50
NF = 5
NC = 8
NL = N // NC              # 64_000 nodes per core
DL = NUM_DAGS // NC       # 640 dags per core
OL = NUM_OBS // NC        # 32 obs per core

_F32_MIN = np.float32(np.finfo(np.float32).min)


def _mlp(x, layers):
    for W, b in layers[:-1]:
        x = jax.nn.relu(x @ W + b)
    W, b = layers[-1]
    return x @ W + b


@jax.jit
def _prep(x_c, layers):
    return _mlp(x_c, layers)


@jax.jit
def _msg(h_c, layers):
    return _mlp(h_c, layers)


@jax.jit
def _gather_rows(table, idx):
    return jnp.take(table, idx, axis=0)


@jax.jit
def _cumsum_mm(msgs):
    # blocked inclusive cumsum via triangular matmuls (no scan op)
    B = msgs.shape[0] // 128
    m3 = msgs.reshape(B, 128, DIM).astype(jnp.float32)
    tri = jnp.tril(jnp.ones((128, 128), jnp.float32))
    cb = jnp.einsum("ij,bjf->bif", tri, m3)
    bs = m3.sum(axis=1)                                # [B, DIM]
    tri_b = jnp.tril(jnp.ones((B, B), jnp.float32), k=-1)
    off = tri_b @ bs                                   # exclusive block offsets
    C = (cb + off[:, None, :]).reshape(B * 128, DIM)
    return jnp.concatenate([jnp.zeros((1, DIM), jnp.float32), C], axis=0)


@jax.jit
def _bound_update(h_c, Cz, bnd, layers):
    G = jnp.take(Cz, bnd, axis=0)                      # [NL+1, DIM]
    agg = G[1:] - G[:-1]
    deg = bnd[1:] - bnd[:-1]
    nmask = (deg > 0).astype(h_c.dtype)[:, None]
    return h_c + nmask * _mlp(agg, layers)


@jax.jit
def _final(x_c, h_c, stage_mask_c, exec_mask_c, job_local, params):
    dag_in = jnp.concatenate([x_c, h_c], axis=1)
    dmsg = _mlp(dag_in, params["dag_msg"])
    dag_sum = dmsg.reshape(DL, NODES_PER_DAG, DIM).sum(axis=1)
    gmsg = _mlp(dag_sum, params["glob_msg"])
    glob = gmsg.reshape(OL, DAGS_PER_OBS, DIM).sum(axis=1)

    node_in = jnp.concatenate(
        [
            x_c,
            h_c,
            jnp.repeat(dag_sum, NODES_PER_DAG, axis=0),
            jnp.repeat(glob, NL // OL, axis=0),
        ],
        axis=1,
    )
    s = _mlp(node_in, params["node_score"])[:, 0]
    sm = jnp.where(stage_mask_c, s, _F32_MIN)
    local_max = sm.max()

    dag_feat = x_c[job_local * NODES_PER_DAG, 0:3]            # [OL, 3]
    merged = jnp.concatenate([dag_feat, dag_sum[job_local]], axis=1)
    merged_rep = jnp.repeat(merged, NUM_EXEC, axis=0)
    glob_rep = jnp.repeat(glob, NUM_EXEC, axis=0)
    exec_act = jnp.tile(jnp.arange(NUM_EXEC, dtype=x_c.dtype) / NUM_EXEC, OL)[:, None]
    dag_in2 = jnp.concatenate([merged_rep, glob_rep, exec_act], axis=1)
    ds = _mlp(dag_in2, params["dag_score"])[:, 0].reshape(OL, NUM_EXEC)
    dsm = jnp.where(exec_mask_c, ds, _F32_MIN)
    dag_probs_c = jax.nn.softmax(dsm, axis=-1)
    return sm, local_max, dag_probs_c


@jax.jit
def _exp_sum(sm, gmax):
    e = jnp.exp(sm - gmax)
    return e, e.sum()


@jax.jit
def _div(e, z):
    return e / z


def _tree_np(p):
    return jax.tree.map(lambda a: np.asarray(a), p)


def kernel(x, edge_index, edge_mask_batch, ptr, obs_ptr, job_indices,
           stage_mask, exec_mask, params):
    x = np.asarray(x, np.float32)
    edge_index = np.asarray(edge_index, np.int32)
    edge_mask_batch = np.asarray(edge_mask_batch, bool)
    job_indices = np.asarray(job_indices, np.int32)
    stage_mask = np.asarray(stage_mask, bool)
    exec_mask = np.asarray(exec_mask, bool)
    params = _tree_np(params)

    devs = jax.devices()[:NC]

    # ---- host-side index prep: shard edges by src core, sort by src, compact
    src, dst = edge_index[0], edge_index[1]
    core_of = src // NL
    dst_lists = [[None] * DEPTH for _ in range(NC)]
    bnd_lists = [[None] * DEPTH for _ in range(NC)]
    emax = 0
    for t in range(DEPTH):
        act = edge_mask_batch[t]
        for c in range(NC):
            sel = act & (core_of == c)
            s_l = src[sel] - c * NL
            d_l = dst[sel]
            order = np.argsort(s_l, kind="stable")
            dst_lists[c][t] = d_l[order].astype(np.int32)
            counts = np.bincount(s_l, minlength=NL)
            bnd = np.zeros(NL + 1, np.int32)
            np.cumsum(counts, out=bnd[1:])
            bnd_lists[c][t] = bnd
            emax = max(emax, len(dst_lists[c][t]))
    emax = ((emax + 127) // 128) * 128
    for c in range(NC):
        for t in range(DEPTH):
            d = dst_lists[c][t]
            pad = np.zeros(emax, np.int32)
            pad[: len(d)] = d
            dst_lists[c][t] = pad

    # ---- ship per-core tensors
    xs = [jax.device_put(x[c * NL:(c + 1) * NL], devs[c]) for c in range(NC)]
    prm = [jax.device_put(params, devs[c]) for c in range(NC)]
    sm_l = [jax.device_put(stage_mask[c * NL:(c + 1) * NL], devs[c]) for c in range(NC)]
    em_l = [jax.device_put(exec_mask[c * OL:(c + 1) * OL], devs[c]) for c in range(NC)]
    job_l = [
        jax.device_put((job_indices[c * OL:(c + 1) * OL] - c * DL).astype(np.int32), devs[c])
        for c in range(NC)
    ]
    dsts = [[jax.device_put(dst_lists[c][t], devs[c]) for t in range(DEPTH)] for c in range(NC)]
    bnds = [[jax.device_put(bnd_lists[c][t], devs[c]) for t in range(DEPTH)] for c in range(NC)]

    # ---- node prep
    hs = [_prep(xs[c], prm[c]["node_prep"]) for c in range(NC)]

    # ---- message passing depths
    for t in range(DEPTH):
        ys = [_msg(hs[c], prm[c]["node_msg"]) for c in range(NC)]
        y_full = np.concatenate([np.asarray(y) for y in ys], axis=0)
        yf = [jax.device_put(y_full, devs[c]) for c in range(NC)]
        msgs = [_gather_rows(yf[c], dsts[c][t]) for c in range(NC)]
        czs = [_cumsum_mm(msgs[c]) for c in range(NC)]
        hs = [
            _bound_update(hs[c], czs[c], bnds[c][t], prm[c]["node_update"])
            for c in range(NC)
        ]

    # ---- final phases
    sms, lmaxes, dps = [], [], []
    for c in range(NC):
        sm, lm, dp = _final(xs[c], hs[c], sm_l[c], em_l[c], job_l[c], prm[c])
        sms.append(sm)
        lmaxes.append(lm)
        dps.append(dp)
    gmax = np.max([np.asarray(m) for m in lmaxes]).astype(np.float32)
    es, zs = [], []
    for c in range(NC):
        e, z = _exp_sum(sms[c], jax.device_put(gmax, devs[c]))
        es.append(e)
        zs.append(z)
    Z = np.sum([np.asarray(z) for z in zs]).astype(np.float32)
    ps = [_div(es[c], jax.device_put(Z, devs[c])) for c in range(NC)]

    node_probs = np.concatenate([np.asarray(p) for p in ps], axis=0)
    dag_probs = np.concatenate([np.asarray(d) for d in dps], axis=0)
    return node_probs.astype(np.float32), dag_probs.astype(np.float32)
